# revision 22
# baseline (speedup 1.0000x reference)
"""Trainium2 Bass kernel for nn_ConditionExplicitattnBlock (dense transformer block:
cross-attention + self-attention + MLP, B=2, S=2048, L=1024, D=1024, H=16, DF=4096).

Sharding: 8 cores = 2 batches x 4-way split of the 2048 query rows (512 rows/core).
Cross-attention K/V come from `cond` (replicated per batch group). Self-attention
K/V are computed from each core's own 512 rows and AllGathered (bf16, one fused
collective) within each 4-core batch group. Everything else is row-local.

On-chip layout: activations are kept feature-major ("xT": [channel, token]) so every
matmul (out = lhsT.T @ rhs) needs no activation transposes:
  - projections:  lhsT = host-pre-transposed weight tile, rhs = xT            -> yT
  - scores^T:     lhsT = kT head-slice [64, 128],  rhs = qT head-slice        -> [k, q]
  - softmax:      exp() on ScalarE straight out of PSUM (scores are O(1)-bounded so
                  no max-subtraction is needed); the additive distance bias is
                  applied multiplicatively AFTER the exp: eb = exp(-gamma^2*distT)
                  is precomputed once per key tile (shared by all 16 heads) and
                  pr = exp(scores) * eb on the DVE. This removes the per-head
                  bias-injection matmuls from the PE entirely.
  - attn@V:       lhsT = V_ext [k, 65], rhs = probs^T [k, q]                  -> [d+1, q]
                  (65th V column of ones yields the softmax denominator row)
  - divide:       per head-pair: reciprocal on DVE, K=2 selector matmul broadcasts
                  it over the pair's 128 partitions, one DVE multiply. Pipelines
                  with the next head pair instead of a full-H barrier.
LayerNorm stats (feature dim = partition dim) are computed with ones-vector matmuls
on the PE in float32r (1 cycle/row at N=512, vs 4 for plain fp32); rsqrt uses
exp(-0.5*ln(x)) on ScalarE.
"""

import threading

import numpy as np
import ml_dtypes

import concourse.bass as bass
import concourse.mybir as mybir
import concourse.tile as tile
from concourse import bacc

# ---------------------------------------------------------------- problem dims
B, S, L, D, C, H = 2, 2048, 1024, 1024, 768, 16
HD, DF = 64, 4096
EPS = 1e-6
NCORES, GROUP = 8, 4
SQ = S // GROUP          # 512 query rows per core
P = 128
DC = D // P              # 8  channel chunks of D
CC = C // P              # 6  channel chunks of C
FC = DF // P             # 32 channel chunks of DF
KC_SA = S // P           # 16 key chunks (self-attn)
KC_CA = L // P           # 8  key chunks (cross-attn)
HE = HD + 1              # 65: head dim + ones column

F32 = mybir.dt.float32
F32R = mybir.dt.float32r
BF16 = mybir.dt.bfloat16
AF = mybir.ActivationFunctionType
ALU = mybir.AluOpType
BF16NP = ml_dtypes.bfloat16


def build_bass(with_collective=True, apply_lnb=False):
    nc = bacc.Bacc("TRN2", target_bir_lowering=False, debug=False,
                   num_devices=NCORES)

    di = lambda name, shape, dt=BF16: nc.dram_tensor(name, shape, dt, kind="ExternalInput")
    xT_d = di("xT", [D, SQ], F32)
    condT_d = di("condT", [C, L])
    sa_dist_d = di("sa_dist", [SQ, S])
    ca_dist_d = di("ca_dist", [SQ, L])
    # g_* carry -gamma^2, precomputed host-side
    g_sa_d = di("g_sa", [1, 1], F32)
    g_ca_d = di("g_ca", [1, 1], F32)
    # ln weights are folded into the following projection weights host-side;
    # ln biases are all-zero in this problem (apply_lnb=True compiles the
    # general per-channel bias pass as a fallback).
    ln_d = ({k: di(k, [P, DC], F32) for k in ("ln1_b", "ln2_b", "ln3_b")}
            if apply_lnb else {})
    wq_ca_d = di("wq_ca", [D, D])
    wk_ca_d = di("wk_ca", [C, D])
    wv_ca_d = di("wv_ca", [C, D])
    wo_ca_d = di("wo_ca", [D, D])
    wq_sa_d = di("wq_sa", [D, D])
    wk_sa_d = di("wk_sa", [D, D])
    wv_sa_d = di("wv_sa", [D, D])
    wo_sa_d = di("wo_sa", [D, D])
    w1_d = di("w1T", [D, DF])
    w2_d = di("w2T", [DF, D])
    b1_d = di("b1r", [P, FC], F32)
    b2_d = di("b2r", [P, DC], F32)
    sel2_d = di("sel2", [1, 2 * P])
    out_d = nc.dram_tensor("outT", [D, SQ], F32, kind="ExternalOutput")
    out_re = out_d.rearrange("(dc p) s -> p dc s", p=P)

    with tile.TileContext(nc) as tc:
        with (
            tc.tile_pool(name="const", bufs=1) as cst,
            tc.tile_pool(name="pers", bufs=1) as pers,
            tc.tile_pool(name="probsp", bufs=4) as probsp,
            tc.tile_pool(name="smalls", bufs=2) as smalls,
            tc.tile_pool(name="dram", bufs=1, space="DRAM") as dram,
        ):
            # ------------------------------------------------ residual stream
            # xT DMA first: LN1 consumes it chunk-by-chunk as it lands.
            xT = pers.tile([P, DC, SQ], F32)        # residual stream (in-place)
            xT_re = xT_d.rearrange("(dc p) s -> p dc s", p=P)
            for dc in range(DC):
                nc.sync.dma_start(xT[:, dc], xT_re[:, dc])

            # ------------------------------------------------ constants / params
            ones_f = cst.tile([P, 1], BF16)
            nc.gpsimd.memset(ones_f[:], 1.0)
            ones_row = cst.tile([1, P], F32)
            nc.gpsimd.memset(ones_row[:], 1.0)
            eps_t = cst.tile([P, 1], F32)
            nc.gpsimd.memset(eps_t[:], EPS)
            # [1, 2P] selector row: cols 0:P -> even head (partitions 0:64),
            # cols P:2P -> odd head (partitions 64:128)
            sel2 = cst.tile([1, 2 * P], BF16)
            nc.sync.dma_start(sel2[:], sel2_d[:])
            # -gamma^2 broadcast across partitions (for the eb activations)
            g2B = {}
            for nm, gd in (("sa", g_sa_d), ("ca", g_ca_d)):
                gt = cst.tile([P, 1], F32, name=f"g2B_{nm}")
                nc.sync.dma_start(gt[:], gd[:].to_broadcast([P, 1]))
                g2B[nm] = gt
            lnp = {}
            for k, t in ln_d.items():
                lt = cst.tile([P, DC], F32, name=k)
                nc.sync.dma_start(lt[:], t[:])
                lnp[k] = lt
            b1r = cst.tile([P, FC], F32)
            nc.sync.dma_start(b1r[:], b1_d[:])
            b2r = cst.tile([P, DC], F32)
            nc.sync.dma_start(b2r[:], b2_d[:])

            # ------------------------------------------------ persistent activations
            xn = pers.tile([P, DC, SQ], BF16)       # LN output / attn-out (reused)
            scr = pers.tile([P, DC, SQ], F32)       # x^2 / raw attn@V / mlp out

            # ------------------------------------------------ helpers
            def layer_norm(bt=None):
                """xT (f32) -> xn (bf16), normalized over the channel dim.

                xn = x * rsigB - (mu*rsig)B. The [1,SQ] stats are broadcast
                across partitions with a K=1 ones-matmul into PSUM (no DRAM
                round-trip). Stats matmuls run on bf16 casts of x and x^2
                (1 cycle/row vs 4 for fp32; the f32 PSUM accumulation keeps
                the mean/var error ~1e-4). ln weight is pre-folded into the
                next projection weights host-side; bias optional per channel."""
                with tc.tile_pool(name="lnsc", bufs=1) as lnsc:
                    with tc.tile_pool(name="lnps", bufs=1, space="PSUM") as lnps:
                        xb = lnsc.tile([P, DC, SQ], BF16, tag="xb")
                        sqb = lnsc.tile([P, DC, SQ], BF16, tag="sqb")
                        mu_ps = lnps.tile([1, SQ], F32, tag="mu")
                        m2_ps = lnps.tile([1, SQ], F32, tag="m2")
                        for dc in range(DC):
                            nc.gpsimd.tensor_copy(xb[:, dc], xT[:, dc])
                            nc.scalar.square(sqb[:, dc], xT[:, dc])
                            nc.tensor.matmul(mu_ps[:], ones_f[:], xb[:, dc],
                                             start=(dc == 0), stop=(dc == DC - 1))
                        for dc in range(DC):
                            nc.tensor.matmul(m2_ps[:], ones_f[:], sqb[:, dc],
                                             start=(dc == 0), stop=(dc == DC - 1))
                        mu = lnsc.tile([1, SQ], F32, tag="mu")
                        nc.vector.tensor_scalar_mul(mu[:], mu_ps[:], 1.0 / D)
                        mu2 = lnsc.tile([1, SQ], F32, tag="mu2")
                        nc.vector.tensor_mul(mu2[:], mu[:], mu[:])
                        # var = m2/D - mu^2 (fused; one PSUM operand)
                        var = lnsc.tile([1, SQ], F32, tag="var")
                        nc.vector.scalar_tensor_tensor(
                            var[:], m2_ps[:], 1.0 / D, mu2[:],
                            ALU.mult, ALU.subtract)
                        # rsig = exp(-0.5 * ln(var + eps))
                        nc.scalar.activation(var[:], var[:], AF.Ln, bias=eps_t[0:1])
                        rsig = lnsc.tile([1, SQ], F32, tag="rsig")
                        nc.scalar.activation(rsig[:], var[:], AF.Exp, scale=-0.5)
                        ms = lnsc.tile([1, SQ], F32, tag="ms")
                        nc.vector.tensor_mul(ms[:], mu[:], rsig[:])
                        rs_ps = lnps.tile([P, SQ], F32, tag="rsB")
                        ms_ps = lnps.tile([P, SQ], F32, tag="msB")
                        nc.tensor.matmul(rs_ps[:], ones_row[:], rsig[:],
                                         start=True, stop=True)
                        nc.tensor.matmul(ms_ps[:], ones_row[:], ms[:],
                                         start=True, stop=True)
                        # evacuate the broadcasts to SBUF so the PSUM pool
                        # frees before the normalize finishes (the next
                        # phase's PSUM tiles reuse these banks)
                        rs_sb = lnsc.tile([P, SQ], F32, tag="rs_sb")
                        nc.vector.tensor_copy(rs_sb[:], rs_ps[:])
                        ms_sb = lnsc.tile([P, SQ], F32, tag="ms_sb")
                        nc.vector.tensor_copy(ms_sb[:], ms_ps[:])
                    # per-chunk (mult on Pool, sub on DVE) so downstream
                    # matmuls start on chunk 0 while later chunks normalize
                    for dc in range(DC):
                        nc.gpsimd.tensor_tensor(scr[:, dc], xT[:, dc],
                                                rs_sb[:], ALU.mult)
                        nc.vector.tensor_tensor(xn[:, dc], scr[:, dc],
                                                ms_sb[:], ALU.subtract)
                if bt is not None:
                    for dc in range(DC):
                        nc.vector.tensor_scalar_add(xn[:, dc], xn[:, dc],
                                                    bt[:, dc:dc + 1])

            def proj_fm(x_bf, w_dram, IC, OC, wpool, pp, evict):
                """Feature-major projection: out[:, oc] = w[:, :, oc].T @ x."""
                wre = w_dram.rearrange("(ic p) o -> p ic o", p=P)
                for oc in range(OC):
                    wsb = wpool.tile([P, IC, P], BF16, tag="w")
                    nc.sync.dma_start(wsb[:], wre[:, :, oc * P:(oc + 1) * P])
                    pt = pp.tile([P, SQ], F32, tag="pp")
                    for ic in range(IC):
                        nc.tensor.matmul(pt[:], wsb[:, ic], x_bf[:, ic],
                                         start=(ic == 0), stop=(ic == IC - 1))
                    evict(oc, pt)

            def attention(q_sb, k_sb, KC, vext, ebT, out_bf):
                # Heads processed in pairs: the even head lives on partitions
                # 0:64 of channel chunk dc, the odd head on 64:128. The distance
                # bias (shared across heads) is applied as pr = exp(scores)*eb
                # on the DVE. Each pair's softmax division runs right after its
                # attn@V accumulation, pipelined with the next pair.
                with (
                    tc.tile_pool(name="scps", bufs=2, space="PSUM") as scps,
                    tc.tile_pool(name="avps", bufs=2, space="PSUM") as avps,
                    tc.tile_pool(name="divps", bufs=2, space="PSUM") as divps,
                ):
                    for dc in range(H // 2):
                        he, ho = 2 * dc, 2 * dc + 1
                        av_e = avps.tile([HE, SQ], F32, tag="av")
                        av_o = avps.tile([HE, SQ], F32, tag="av")
                        for kc in range(KC):
                            ks = k_sb[:, dc, kc * P:(kc + 1) * P]
                            sp = scps.tile([P, 2, SQ], F32, tag="sc")
                            nc.tensor.matmul(sp[:, 0], ks[0:64], q_sb[0:64, dc],
                                             start=True, stop=True)
                            nc.tensor.matmul(sp[:, 1], ks[64:128], q_sb[64:128, dc],
                                             start=True, stop=True)
                            pr = probsp.tile([P, 2, SQ], BF16, tag="pr")
                            nc.scalar.activation(pr[:], sp[:], AF.Exp)
                            nc.vector.tensor_tensor(
                                pr[:], pr[:],
                                ebT[:, kc][:, None, :].to_broadcast([P, 2, SQ]),
                                ALU.mult)
                            nc.tensor.matmul(av_e[:], vext[:, kc, he * HE:(he + 1) * HE],
                                             pr[:, 0], start=(kc == 0), stop=(kc == KC - 1))
                            nc.tensor.matmul(av_o[:], vext[:, kc, ho * HE:(ho + 1) * HE],
                                             pr[:, 1], start=(kc == 0), stop=(kc == KC - 1))
                        # per-pair softmax division (row 64 = denominator):
                        # reciprocal straight off the PSUM row, bf16-cast on
                        # Pool, then two accumulating K=1 selector matmuls
                        # broadcast the pair over the 128 partitions.
                        rec_e = smalls.tile([1, SQ], F32, tag="rec_e")
                        nc.vector.reciprocal(rec_e[:], av_e[64:65])
                        rec_o = smalls.tile([1, SQ], F32, tag="rec_o")
                        nc.vector.reciprocal(rec_o[:], av_o[64:65])
                        rec_eb = smalls.tile([1, SQ], BF16, tag="rec_eb")
                        nc.gpsimd.tensor_copy(rec_eb[:], rec_e[:])
                        rec_ob = smalls.tile([1, SQ], BF16, tag="rec_ob")
                        nc.gpsimd.tensor_copy(rec_ob[:], rec_o[:])
                        rb_ps = divps.tile([P, SQ], F32, tag="rb")
                        nc.tensor.matmul(rb_ps[:], sel2[:, 0:P], rec_eb[:],
                                         start=True, stop=False)
                        nc.tensor.matmul(rb_ps[:], sel2[:, P:2 * P], rec_ob[:],
                                         start=False, stop=True)
                        nc.vector.tensor_copy(scr[0:64, dc, :], av_e[0:64, :])
                        nc.vector.tensor_copy(scr[64:128, dc, :], av_o[0:64, :])
                        nc.vector.tensor_tensor(out_bf[:, dc], scr[:, dc],
                                                rb_ps[:], ALU.mult)

            def o_proj_residual(av_bf, w_dram, wpool, pp):
                wre = w_dram.rearrange("(ic p) o -> p ic o", p=P)
                for oc in range(DC):
                    wsb = wpool.tile([P, DC, P], BF16, tag="w")
                    nc.sync.dma_start(wsb[:], wre[:, :, oc * P:(oc + 1) * P])
                    pt = pp.tile([P, SQ], F32, tag="pp")
                    for ic in range(DC):
                        nc.tensor.matmul(pt[:], wsb[:, ic], av_bf[:, ic],
                                         start=(ic == 0), stop=(ic == DC - 1))
                    nc.vector.tensor_add(xT[:, oc], xT[:, oc], pt[:])

            # ============================================================ CA ====
            with tc.tile_pool(name="ca", bufs=1) as ca, \
                 tc.tile_pool(name="caw", bufs=4) as caw:
                condT = ca.tile([P, CC, L], BF16)
                nc.sync.dma_start(condT[:],
                                  condT_d.rearrange("(cc p) l -> p cc l", p=P))
                # eb = exp(-g^2 * distT), shared across all heads
                ebT_ca = ca.tile([P, KC_CA, SQ], BF16)
                for kc in range(KC_CA):
                    nc.sync.dma_start_transpose(
                        ebT_ca[:, kc], ca_dist_d[:, kc * P:(kc + 1) * P])
                layer_norm(lnp.get("ln1_b"))
                nc.scalar.activation(ebT_ca[:], ebT_ca[:], AF.Exp,
                                     scale=g2B["ca"][:])
                qca = ca.tile([P, DC, SQ], BF16)
                kca = ca.tile([P, DC, L], BF16)
                vca = ca.tile([P, KC_CA, H * HE], BF16)
                nc.gpsimd.memset(
                    vca[:].rearrange("p k (h e) -> p k h e", e=HE)[:, :, :, 64:65],
                    1.0)

                with tc.tile_pool(name="pca", bufs=2, space="PSUM") as pp:
                    proj_fm(xn, wq_ca_d, DC, DC, caw, pp,
                            lambda oc, pt: nc.vector.tensor_copy(qca[:, oc], pt[:]))
                    # kT: [d, l] = wk.T @ condT; N = L = 1024 -> two 512 halves
                    wkre = wk_ca_d.rearrange("(ic p) o -> p ic o", p=P)
                    for oc in range(DC):
                        wsb = caw.tile([P, CC, P], BF16, tag="w")
                        nc.sync.dma_start(wsb[:], wkre[:, :, oc * P:(oc + 1) * P])
                        for nq in range(2):
                            pt = pp.tile([P, SQ], F32, tag="pp")
                            for ic in range(CC):
                                nc.tensor.matmul(
                                    pt[:], wsb[:, ic],
                                    condT[:, ic, nq * SQ:(nq + 1) * SQ],
                                    start=(ic == 0), stop=(ic == CC - 1))
                            nc.vector.tensor_copy(
                                kca[:, oc, nq * SQ:(nq + 1) * SQ], pt[:])
                    # V (seq-major, strided into vca with ones cols kept)
                    wvca = ca.tile([P, CC, D], BF16)
                    nc.sync.dma_start(wvca[:],
                                      wv_ca_d.rearrange("(ic p) o -> p ic o", p=P))
                    for lc in range(KC_CA):
                        for nd in range(2):
                            pt = pp.tile([P, SQ], F32, tag="pp")
                            for ic in range(CC):
                                nc.tensor.matmul(
                                    pt[:], condT[:, ic, lc * P:(lc + 1) * P],
                                    wvca[:, ic, nd * SQ:(nd + 1) * SQ],
                                    start=(ic == 0), stop=(ic == CC - 1))
                            dst = vca[:, lc].rearrange(
                                "p (h e) -> p h e", e=HE)[:, nd * 8:(nd + 1) * 8, 0:64]
                            nc.vector.tensor_copy(
                                dst, pt[:].rearrange("p (h e) -> p h e", e=64))

                attention(qca, kca, KC_CA, vca, ebT_ca, xn)
                with tc.tile_pool(name="poc", bufs=2, space="PSUM") as pp2:
                    o_proj_residual(xn, wo_ca_d, caw, pp2)

            # ============================================================ SA ====
            layer_norm(lnp.get("ln2_b"))
            k_bounce = dram.tile([P, DC * SQ], BF16)
            k_gath = dram.tile([GROUP, P, DC * SQ], BF16)
            v_bounce = dram.tile([P, 4 * D], BF16)
            v_gath = dram.tile([GROUP, P, 4 * D], BF16)
            with tc.tile_pool(name="sa", bufs=1) as sa, \
                 tc.tile_pool(name="saw", bufs=4) as saw, \
                 tc.tile_pool(name="stg", bufs=1) as stg:
                qsa = sa.tile([P, DC, SQ], BF16)
                kg = sa.tile([P, DC, S], BF16)
                ebT_sa = sa.tile([P, KC_SA, SQ], BF16)
                for kc in range(KC_SA):
                    nc.sync.dma_start_transpose(
                        ebT_sa[:, kc], sa_dist_d[:, kc * P:(kc + 1) * P])
                nc.scalar.activation(ebT_sa[:], ebT_sa[:], AF.Exp,
                                     scale=g2B["sa"][:])
                # prefetch the big seq-major V weight before the K projection
                wvsa = sa.tile([P, DC, D], BF16)
                nc.sync.dma_start(wvsa[:],
                                  wv_sa_d.rearrange("(ic p) o -> p ic o", p=P))
                vext = sa.tile([P, KC_SA, H * HE], BF16)
                nc.gpsimd.memset(
                    vext[:].rearrange("p k (h e) -> p k h e", e=HE)[:, :, :, 64:65],
                    1.0)
                rg = [[0, 1, 2, 3], [4, 5, 6, 7]]
                with tc.tile_pool(name="psa", bufs=2, space="PSUM") as pp:
                    # own K first -> stage -> bounce -> AllGather ASAP
                    kstage = stg.tile([P, DC, SQ], BF16, tag="stage")
                    proj_fm(xn, wk_sa_d, DC, DC, saw, pp,
                            lambda oc, pt: nc.vector.tensor_copy(kstage[:, oc], pt[:]))
                    nc.sync.dma_start(k_bounce[:], kstage[:].rearrange("p a b -> p (a b)"))
                    if with_collective:
                        nc.gpsimd.collective_compute(
                            "AllGather", ALU.bypass,
                            ins=[k_bounce.opt()], outs=[k_gath.opt()],
                            replica_groups=rg)
                    else:
                        for r in range(GROUP):
                            nc.sync.dma_start(k_gath[r], k_bounce[:])
                    # own V (seq-major) -> stage -> bounce -> AllGather
                    vstage = stg.tile([P, 4, D], BF16, tag="stage")
                    for sc in range(4):
                        for nd in range(2):
                            pt = pp.tile([P, SQ], F32, tag="pp")
                            for ic in range(DC):
                                nc.tensor.matmul(
                                    pt[:], xn[:, ic, sc * P:(sc + 1) * P],
                                    wvsa[:, ic, nd * SQ:(nd + 1) * SQ],
                                    start=(ic == 0), stop=(ic == DC - 1))
                            nc.vector.tensor_copy(
                                vstage[:, sc, nd * SQ:(nd + 1) * SQ], pt[:])
                    nc.sync.dma_start(v_bounce[:], vstage[:].rearrange("p a b -> p (a b)"))
                    if with_collective:
                        nc.gpsimd.collective_compute(
                            "AllGather", ALU.bypass,
                            ins=[v_bounce.opt()], outs=[v_gath.opt()],
                            replica_groups=rg)
                    else:
                        for r in range(GROUP):
                            nc.sync.dma_start(v_gath[r], v_bounce[:])
                    # Q overlaps the collectives
                    proj_fm(xn, wq_sa_d, DC, DC, saw, pp,
                            lambda oc, pt: nc.vector.tensor_copy(qsa[:, oc], pt[:]))

                for r in range(GROUP):
                    nc.sync.dma_start(
                        kg[:, :, r * SQ:(r + 1) * SQ],
                        k_gath[r].rearrange("p (dc s) -> p dc s", s=SQ))
                    for sc in range(4):
                        src = v_gath[r].rearrange(
                            "p (sc d) -> p sc d", d=D)[:, sc].rearrange(
                            "p (h e) -> p h e", e=64)
                        dst = vext[:, r * 4 + sc].rearrange(
                            "p (h e) -> p h e", e=HE)[:, :, 0:64]
                        nc.sync.dma_start(dst, src)

                attention(qsa, kg, KC_SA, vext, ebT_sa, xn)
                with tc.tile_pool(name="pos", bufs=2, space="PSUM") as pp2:
                    o_proj_residual(xn, wo_sa_d, saw, pp2)

            # =========================================================== MLP ====
            layer_norm(lnp.get("ln3_b"))
            with tc.tile_pool(name="mlp", bufs=1) as mlp, \
                 tc.tile_pool(name="w1p", bufs=4) as w1p, \
                 tc.tile_pool(name="w2p", bufs=3) as w2p:
                h_bf = mlp.tile([P, FC, SQ], BF16)
                w1re = w1_d.rearrange("(ic p) o -> p ic o", p=P)
                w2re = w2_d.rearrange("(f p) o -> p f o", p=P)
                with tc.tile_pool(name="pm1", bufs=2, space="PSUM") as pp:
                    for fc in range(FC):
                        wsb = w1p.tile([P, DC, P], BF16, tag="w1")
                        nc.sync.dma_start(wsb[:], w1re[:, :, fc * P:(fc + 1) * P])
                        pt = pp.tile([P, SQ], F32, tag="pp")
                        for ic in range(DC):
                            nc.tensor.matmul(pt[:], wsb[:, ic], xn[:, ic],
                                             start=(ic == 0), stop=(ic == DC - 1))
                        nc.scalar.activation(h_bf[:, fc], pt[:], AF.Gelu,
                                             bias=b1r[:, fc:fc + 1])
                    for oc in range(DC):
                        wsb = w2p.tile([P, FC, P], BF16, tag="w2")
                        nc.sync.dma_start(wsb[:], w2re[:, :, oc * P:(oc + 1) * P])
                        pt = pp.tile([P, SQ], F32, tag="pp")
                        for fc in range(FC):
                            nc.tensor.matmul(pt[:], wsb[:, fc], h_bf[:, fc],
                                             start=(fc == 0), stop=(fc == FC - 1))
                        nc.vector.tensor_add(scr[:, oc], xT[:, oc], pt[:])
                        nc.vector.tensor_scalar_add(scr[:, oc], scr[:, oc],
                                                    b2r[:, oc:oc + 1])
                        nc.sync.dma_start(out_re[:, oc], scr[:, oc])

    nc.compile()
    return nc


# ---------------------------------------------------------------- host wrapper
_cache = {}
_lock = threading.Lock()


def _get_nc():
    with _lock:
        if "nc" not in _cache:
            _cache["nc"] = build_bass()
        return _cache["nc"]


def _prep_in_maps(x, cond, sa_distance_matrix, ca_distance_matrix,
                  gamma_ca, gamma_sa,
                  ln1_w, ln1_b, ln2_w, ln2_b, ln3_w, ln3_b,
                  ca_wq, ca_wk, ca_wv, ca_wo, sa_wq, sa_wk, sa_wv, sa_wo,
                  mlp_w1, mlp_b1, mlp_w2, mlp_b2):
    bf = lambda a: np.ascontiguousarray(a).astype(BF16NP)
    f32 = lambda a: np.ascontiguousarray(a, dtype=np.float32)
    scale = 1.0 / np.sqrt(HD)
    w1, w2, w3 = (f32(ln1_w)[:, None], f32(ln2_w)[:, None], f32(ln3_w)[:, None])

    # ln weights fold into the next projections' input dim (wT rows)
    shared = dict(
        g_sa=np.float32(-(np.float32(gamma_sa) ** 2)).reshape(1, 1),
        g_ca=np.float32(-(np.float32(gamma_ca) ** 2)).reshape(1, 1),
        wq_ca=bf((ca_wq * scale).T * w1), wk_ca=bf(ca_wk.T), wv_ca=bf(ca_wv.T),
        wo_ca=bf(ca_wo.T),
        wq_sa=bf((sa_wq * scale).T * w2), wk_sa=bf(sa_wk.T * w2),
        wv_sa=bf(sa_wv.T * w2), wo_sa=bf(sa_wo.T),
        w1T=bf(mlp_w1.T * w3), w2T=bf(mlp_w2.T),
        b1r=f32(mlp_b1).reshape(FC, P).T.copy(),
        b2r=f32(mlp_b2).reshape(DC, P).T.copy(),
    )
    sel2 = np.zeros((1, 2 * P), np.float32)
    sel2[0, 0:HD] = 1.0
    sel2[0, P + HD:2 * P] = 1.0
    shared["sel2"] = bf(sel2)
    if any(np.any(np.asarray(b) != 0) for b in (ln1_b, ln2_b, ln3_b)):
        raise NotImplementedError(
            "nonzero ln bias: rebuild with build_bass(apply_lnb=True) and pass "
            "ln{1,2,3}_b as [P, DC] inputs")

    in_maps = []
    for core in range(NCORES):
        b, r = core // GROUP, core % GROUP
        q0 = r * SQ
        m = dict(shared)
        m["xT"] = f32(x[b, q0:q0 + SQ, :].T)
        m["condT"] = bf(cond[b].T)
        m["sa_dist"] = bf(sa_distance_matrix[b, q0:q0 + SQ, :])
        m["ca_dist"] = bf(ca_distance_matrix[b, q0:q0 + SQ, :])
        in_maps.append(m)
    return in_maps


def kernel(**inputs):
    from concourse.bass_utils import run_bass_kernel_spmd

    nc = _get_nc()
    in_maps = _prep_in_maps(**inputs)
    res = run_bass_kernel_spmd(nc, in_maps, core_ids=list(range(NCORES)))
    out = np.empty((B, S, D), np.float32)
    for core in range(NCORES):
        b, r = core // GROUP, core % GROUP
        out[b, r * SQ:(r + 1) * SQ, :] = res.results[core]["outT"].T
    return out
